# revision 28
# baseline (speedup 1.0000x reference)
"""Trainium2 Bass kernel for nn_Attention_16612933501287.

Cross-attention block: c:(B=8,N=8,C=512,H=32,W=32), RMSNorm over C, fused
KV projection (512->1024), one query per (batch, head) attending over the
N=8 token axis at each spatial position, then output projection (512->512).

Sharding: data-parallel over B — one batch element per NeuronCore (8 cores).

Per-core dataflow (feature-major: channels on partitions, the 1024 spatial
positions on the free dim):
  host prep : fold g into Wkv; qv = emb[q]@Wq+bq; fold qv and the 1/sqrt(64)
              logit scale into a per-batch matrix Wd (512x8) so attention
              logits come straight out of a matmul; k is never materialized.
  n loop    : DMA c[n]; square (DVE/ACT/GPSIMD); ssq and logits accumulate
              across n into persistent PSUM tiles via one-hot-padded
              stationary weights; vraw = Wv.T@cp -> fp16 in SBUF.
  epilogue  : batched softmax (one Sqrt + one Exp -> only 2 ACT table
              loads); softmax denominator via an exact-fp32 selection
              matmul; w~ = e*r/sums in fp16; per-head replication via
              broadcast DMAs from a DRAM bounce (all issued upfront);
              vw = vraw*w~ (DVE fp16); sum over n via identity-matmul
              PSUM accumulation; output projection + bias; DMA out in
              (C,H,W) layout.
Big matmuls run as float32r (fp32 data, 1 PE cycle/row).
"""

import numpy as np

import concourse.bass as bass
import concourse.bacc as bacc
import concourse.mybir as mybir
import concourse.tile as tile
from concourse.bass_utils import run_bass_kernel_spmd

F32 = mybir.dt.float32
F16 = mybir.dt.float16
F32R = mybir.dt.float32r
AF = mybir.ActivationFunctionType

B, N, C, H, W = 8, 8, 512, 32, 32
NH, HS = 8, 64
P = H * W           # 1024 spatial positions per core
NCC = C // 128      # 4 contraction chunks
EPS = 1e-6


def r32(ap):
    return ap if ap.dtype == F32R else ap.bitcast(F32R)


def build_program():
    nc = bacc.Bacc()

    c_d = nc.declare_dram_parameter("c", [N, C, H, W], F32R, isOutput=False)
    wv_d = nc.declare_dram_parameter("wv", [128, NCC, 512], F32R, isOutput=False)
    # zero-padded logit weights: [k, cc, n, n*8+i] nonzero only at column n*8+i
    wdz_d = nc.declare_dram_parameter("wdz", [128, NCC, N, N * NH], F32R,
                                      isOutput=False)
    oh_d = nc.declare_dram_parameter("onehot", [128, N, N], F32R, isOutput=False)
    sel_d = nc.declare_dram_parameter("sel", [N * NH, NH], F32, isOutput=False)
    r8_d = nc.declare_dram_parameter("r8sel", [NH, 2, NH * NH], F32, isOutput=False)
    s64_d = nc.declare_dram_parameter("sel64", [N * NH, N, NCC, 128], F16,
                                      isOutput=False)
    wo_d = nc.declare_dram_parameter("wout", [128, NCC, 512], F16, isOutput=False)
    id_d = nc.declare_dram_parameter("ident", [128, 128], F16, isOutput=False)
    bo_d = nc.declare_dram_parameter("bout", [128, NCC], F32, isOutput=False)
    out_d = nc.declare_dram_parameter("out", [C, H, W], F32, isOutput=True)

    with tile.TileContext(nc) as tc:
        with (
            tc.tile_pool(name="consts", bufs=1) as consts,
            tc.tile_pool(name="store", bufs=1) as store,
            tc.tile_pool(name="smalls", bufs=1) as smalls,
            tc.tile_pool(name="osb_pool", bufs=2) as osb_pool,
            tc.tile_pool(name="ps_stat", bufs=1, space="PSUM") as ps_stat,
            tc.tile_pool(name="ps_big", bufs=2, space="PSUM") as ps_big,
        ):
            # loop-critical consts first (tiny oh so PE can start early);
            # wv/wdz loads are emitted inside n=0 after the first cp chunks,
            # epilogue-only weights after the loop.
            wdz_sb = consts.tile([128, NCC, N, N * NH], F32R)
            nc.sync.dma_start(out=wdz_sb[:, 0], in_=wdz_d[:, 0])
            wv_sb = consts.tile([128, NCC, 512], F32R)
            nc.sync.dma_start(out=wv_sb[:, 0], in_=wv_d[:, 0])
            oh_sb = consts.tile([128, N, N], F32R)
            nc.sync.dma_start(out=oh_sb, in_=oh_d[:])
            sel_sb = consts.tile([N * NH, NH], F32)
            r8_sb = consts.tile([NH, 2, NH * NH], F32)
            s64_sb = consts.tile([N * NH, N, NCC, 128], F16)
            wo_sb = consts.tile([128, NCC, 512], F16)
            id_sb = consts.tile([128, 128], F16)
            bo_sb = consts.tile([128, NCC], F32)

            # persistent accumulators / stores
            vraw_all = store.tile([128, N, NCC, P], F16)   # 8 MiB
            o_sb = store.tile([128, NCC, P], F16)
            ssq_ps = ps_stat.tile([N, P], F32)             # 2 banks, whole loop
            draw_ps = ps_stat.tile([N * NH, P], F32)       # 2 banks, whole loop

            # ================= main loop over token index n =================
            cp_ctx = tc.tile_pool(name="cp_pool", bufs=2)
            cp_pool = cp_ctx.__enter__()
            sq_ctx = tc.tile_pool(name="sq_pool", bufs=2)
            sq_pool = sq_ctx.__enter__()
            for n in range(N):
                cp = cp_pool.tile([128, NCC, P], F32R)
                if n == 0:
                    # per-cc loads interleaved with the weights they unblock
                    for cc in range(NCC):
                        nc.sync.dma_start(
                            out=cp[:, cc, :],
                            in_=c_d[:].rearrange(
                                "n (cc k) h w -> n cc k (h w)", k=128)[n, cc],
                        )
                        if cc < NCC - 1:
                            nc.sync.dma_start(out=wdz_sb[:, cc + 1],
                                              in_=wdz_d[:, cc + 1])
                            nc.sync.dma_start(out=wv_sb[:, cc + 1],
                                              in_=wv_d[:, cc + 1])
                else:
                    for half in range(2):
                        nc.sync.dma_start(
                            out=cp[:, 2 * half:2 * half + 2, :],
                            in_=c_d[:].rearrange(
                                "n (hf cc k) h w -> n hf k cc (h w)",
                                hf=2, k=128)[n, half],
                        )

                def emit_draw(n=n, cp=cp):
                    for cc in range(NCC):
                        for h in range(2):
                            nc.tensor.matmul(
                                draw_ps[:, h * 512:(h + 1) * 512],
                                r32(wdz_sb[:, cc, n, :]),
                                r32(cp[:, cc, h * 512:(h + 1) * 512]),
                                start=(n == 0 and cc == 0),
                                stop=(n == N - 1 and cc == NCC - 1),
                            )

                def emit_vraw(n=n, cp=cp):
                    # cc-outer / h-inner: one weight load serves both halves
                    for ck in range(NCC):
                        v_ps = ps_big.tile([128, P], F32, tag="pair",
                                           name="v_ps")
                        for cc in range(NCC):
                            for h in range(2):
                                nc.tensor.matmul(
                                    v_ps[:, h * 512:(h + 1) * 512],
                                    r32(wv_sb[:, cc, ck * 128:(ck + 1) * 128]),
                                    r32(cp[:, cc, h * 512:(h + 1) * 512]),
                                    start=(cc == 0),
                                    stop=(cc == NCC - 1),
                                )
                        nc.scalar.copy(out=vraw_all[:, n, ck, :], in_=v_ps)

                def emit_ssq(n=n, cp=cp):
                    sq = sq_pool.tile([128, NCC, P], F32R, name="sq")
                    nc.vector.tensor_mul(out=sq[:, 0, :], in0=cp[:, 0, :], in1=cp[:, 0, :])
                    nc.gpsimd.tensor_mul(out=sq[:, 1, :], in0=cp[:, 1, :], in1=cp[:, 1, :])
                    nc.gpsimd.tensor_mul(out=sq[:, 2, :], in0=cp[:, 2, :], in1=cp[:, 2, :])
                    nc.gpsimd.tensor_mul(out=sq[:, 3, :], in0=cp[:, 3, :], in1=cp[:, 3, :])
                    # pre-sum the 4 chunks so ssq needs 2 matmuls/n, not 8
                    sqs = sq_pool.tile([128, P], F32R, name="sqs")
                    nc.vector.tensor_add(out=sqs, in0=sq[:, 0, :], in1=sq[:, 1, :])
                    nc.gpsimd.tensor_add(out=sq[:, 2, :], in0=sq[:, 2, :], in1=sq[:, 3, :])
                    nc.vector.tensor_add(out=sqs, in0=sqs, in1=sq[:, 2, :])
                    for h in range(2):
                        nc.tensor.matmul(
                            ssq_ps[:, h * 512:(h + 1) * 512],
                            r32(oh_sb[:, n, :]),
                            r32(sqs[:, h * 512:(h + 1) * 512]),
                            start=(n == 0),
                            stop=(n == N - 1),
                        )

                if n < N - 2:
                    # stats are epilogue-only: emit them last
                    emit_draw(); emit_vraw(); emit_ssq()
                elif n == N - 2:
                    # defer this vraw until after n=7's stats (loop tail)
                    emit_ssq(); emit_draw()
                    deferred_vraw = emit_vraw
                else:
                    # n=7: stats first, then both deferred vraws — the
                    # softmax chain hides under ~14us of vraw matmuls
                    emit_ssq(); emit_draw()
                    deferred_vraw(); emit_vraw()
            sq_ctx.__exit__(None, None, None)
            cp_ctx.__exit__(None, None, None)

            # ======================== epilogue ========================
            # epilogue-only weights (land during the loop's DMA slack)
            nc.sync.dma_start(out=sel_sb, in_=sel_d[:])
            nc.sync.dma_start(out=r8_sb, in_=r8_d[:])
            nc.sync.dma_start(out=s64_sb, in_=s64_d[:])
            nc.sync.dma_start(out=wo_sb, in_=wo_d[:])
            nc.sync.dma_start(out=id_sb, in_=id_d[:])
            nc.sync.dma_start(out=bo_sb, in_=bo_d[:])

            # r = 1/sqrt(ssq/C + eps)   (one Sqrt: single ACT table load)
            eps_sb = smalls.tile([N, 1], F32)
            nc.vector.memset(eps_sb, EPS)
            rt = smalls.tile([N, P], F32)
            nc.scalar.activation(out=rt, in_=ssq_ps, func=AF.Sqrt,
                                 scale=1.0 / C, bias=eps_sb)
            r_all = rt
            nc.vector.reciprocal_approx_fast(out=r_all, in_=rt)

            # rrep[n*8+i] = r_all[n] via selection matmul (exact fp32)
            rrep = smalls.tile([N * NH, P], F32)
            rr_ps = ps_big.tile([N * NH, P], F32, tag="pair")
            for h in range(2):
                nc.tensor.matmul(rr_ps[:, h * 512:(h + 1) * 512], r8_sb[:, 0, :],
                                 r_all[:, h * 512:(h + 1) * 512],
                                 start=True, stop=True)
            nc.scalar.copy(out=rrep, in_=rr_ps)
            # dots = draw * r ; e = exp(dots)
            e_all = smalls.tile([N * NH, P], F32)
            nc.vector.tensor_mul(out=e_all, in0=draw_ps, in1=rrep)
            nc.scalar.activation(out=e_all, in_=e_all, func=AF.Exp)

            # softmax denominator: sums[i] = sum_n e[n*8+i]
            # exact-fp32 matmul (small, 4 cyc/row is fine)
            rsum = smalls.tile([NH, P], F32)
            s_ps = ps_big.tile([NH, P], F32, tag="pair")
            for h in range(2):
                nc.tensor.matmul(
                    s_ps[:, h * 512:(h + 1) * 512], sel_sb,
                    e_all[:, h * 512:(h + 1) * 512],
                    start=True, stop=True,
                )
            nc.vector.reciprocal_approx_fast(out=rsum, in_=s_ps)
            srep = smalls.tile([N * NH, P], F32)
            sr_ps = ps_big.tile([N * NH, P], F32, tag="pair")
            for h in range(2):
                nc.tensor.matmul(sr_ps[:, h * 512:(h + 1) * 512], r8_sb[:, 1, :],
                                 rsum[:, h * 512:(h + 1) * 512],
                                 start=True, stop=True)
            nc.scalar.copy(out=srep, in_=sr_ps)

            # w~ = e * r / sums  -> fp16, bounce to DRAM for broadcasts
            nc.vector.tensor_mul(out=e_all, in0=e_all, in1=rrep)
            wt = smalls.tile([N * NH, P], F16)
            nc.vector.tensor_mul(out=wt, in0=e_all, in1=srep)

            with (
                tc.tile_pool(name="wrep_pool", bufs=4) as wrep_pool,
                tc.tile_pool(name="vw_pool", bufs=4) as vw_pool,
            ):
                # o = sum_n vraw * w~rep via identity-matmul PSUM accumulation;
                # per-head replication via selection matmuls from wt (on-chip)
                for ck in range(NCC):
                    # o-accumulator reuses the (now idle) stats PSUM banks so
                    # ps_big's 4 slots stay free for the wrep pipeline
                    on_ps = ps_stat.tile(
                        [128, P], F32, name=f"on_ps_{ck}",
                        tag=("ssq_ps" if ck % 2 == 0 else "draw_ps"))
                    for n in range(N):
                        vw = vw_pool.tile([128, P], F16)
                        wr_ps = ps_big.tile([128, P], F32, tag="pair")
                        for h in range(2):
                            nc.tensor.matmul(
                                wr_ps[:, h * 512:(h + 1) * 512],
                                s64_sb[:, n, ck, :],
                                wt[:, h * 512:(h + 1) * 512],
                                start=True, stop=True)
                        nc.vector.tensor_mul(
                            out=vw, in0=vraw_all[:, n, ck, :], in1=wr_ps)
                        for h in range(2):
                            nc.tensor.matmul(
                                on_ps[:, h * 512:(h + 1) * 512],
                                id_sb,
                                vw[:, h * 512:(h + 1) * 512],
                                start=(n == 0),
                                stop=(n == N - 1),
                            )
                    for h in range(2):
                        nc.scalar.copy(
                            out=o_sb[:, ck, h * 512:(h + 1) * 512],
                            in_=on_ps[:, h * 512:(h + 1) * 512]
                        )

                # out = Wout.T @ o + bout
                for do in range(NCC):
                    ot_sb = osb_pool.tile([128, P], F32)
                    ot_ps = ps_big.tile([128, P], F32, tag="pair")
                    for h in range(2):
                        for di in range(NCC):
                            nc.tensor.matmul(
                                ot_ps[:, h * 512:(h + 1) * 512],
                                wo_sb[:, di, do * 128:(do + 1) * 128],
                                o_sb[:, di, h * 512:(h + 1) * 512],
                                start=(di == 0),
                                stop=(di == NCC - 1),
                            )
                    nc.scalar.activation(
                        out=ot_sb, in_=ot_ps,
                        func=AF.Identity, bias=bo_sb[:, do:do + 1],
                    )
                    nc.sync.dma_start(
                        out=out_d[:].rearrange(
                            "(do k) h w -> do k (h w)", k=128)[do],
                        in_=ot_sb,
                    )

    nc.finalize()
    return nc


_CACHE = {}


def _get_nc():
    if "nc" not in _CACHE:
        _CACHE["nc"] = build_program()
    return _CACHE["nc"]


def _prep_inputs(q, c, emb, Wq, bq, Wkv, Wout, bout, g):
    q = np.asarray(q)
    c = np.asarray(c, dtype=np.float32)
    emb = np.asarray(emb, dtype=np.float32)
    Wq = np.asarray(Wq, dtype=np.float32)
    bq = np.asarray(bq, dtype=np.float32)
    Wkv = np.asarray(Wkv, dtype=np.float32)
    Wout = np.asarray(Wout, dtype=np.float32)
    bout = np.asarray(bout, dtype=np.float32)
    g = np.asarray(g, dtype=np.float32)

    qv = emb[q] @ Wq + bq                                   # (B, 512)
    qvs = qv.reshape(B, NH, HS).astype(np.float32) * np.float32(HS ** -0.5)
    Wkv_g = (g[:, None] * Wkv).astype(np.float32)
    Wk3 = Wkv_g[:, :C].reshape(C, NH, HS)
    Wv = np.ascontiguousarray(Wkv_g[:, C:])                 # (512, 512)
    Wd = np.einsum('chs,bhs->bch', Wk3, qvs).astype(np.float32)  # (B, 512, 8)

    wv_host = np.ascontiguousarray(
        Wv.reshape(NCC, 128, 512).transpose(1, 0, 2))       # [k, cc, dv]
    # zero-padded draw weights: [b, k, cc, n, m] = Wd at m = n*8+i
    wdz = np.zeros((B, 128, NCC, N, N * NH), np.float32)
    wd4 = Wd.reshape(B, NCC, 128, NH).transpose(0, 2, 1, 3)  # [b, k, cc, i]
    for n in range(N):
        wdz[:, :, :, n, n * NH:(n + 1) * NH] = wd4
    wout_host = np.ascontiguousarray(
        Wout.reshape(NCC, 128, 512).transpose(1, 0, 2)).astype(np.float16)
    onehot = np.zeros((128, N, N), np.float32)
    for n in range(N):
        onehot[:, n, n] = 1.0
    sel = np.zeros((N * NH, NH), np.float32)
    for n in range(N):
        for i in range(NH):
            sel[n * NH + i, i] = 1.0
    # r8sel[:, 0]: rrep (out row n*8+i <- r row n); r8sel[:, 1]: srep (<- rsum row i)
    r8sel = np.zeros((NH, 2, NH * NH), np.float32)
    for n in range(N):
        for i in range(NH):
            r8sel[n, 0, n * NH + i] = 1.0
            r8sel[i, 1, n * NH + i] = 1.0
    # sel64[kk, n, ck, m] = 1 iff kk == n*8 + 2*ck + m//64
    sel64 = np.zeros((N * NH, N, NCC, 128), np.float16)
    for n in range(N):
        for ck in range(NCC):
            for j in range(2):
                sel64[n * NH + 2 * ck + j, n, ck, j * 64:(j + 1) * 64] = 1.0
    ident = np.eye(128, dtype=np.float16)
    bout_host = np.ascontiguousarray(bout.reshape(NCC, 128).T)  # [k, do]

    in_maps = []
    for b in range(B):
        in_maps.append({
            "c": np.ascontiguousarray(c[b]),
            "wv": wv_host,
            "wdz": np.ascontiguousarray(wdz[b]),
            "onehot": onehot,
            "sel": sel,
            "r8sel": r8sel,
            "sel64": sel64,
            "wout": wout_host,
            "ident": ident,
            "bout": bout_host,
        })
    return in_maps


def kernel(**inputs) -> np.ndarray:
    nc = _get_nc()
    in_maps = _prep_inputs(**inputs)
    res = run_bass_kernel_spmd(nc, in_maps, list(range(B)))
    return np.stack([res.results[b]["out"] for b in range(B)], axis=0)


if __name__ == "__main__":
    nc = build_program()
    print("program built ok")


# revision 29
# speedup vs baseline: 28.2771x; 28.2771x over previous
"""Trainium2 Bass kernel for nn_Attention_16612933501287.

Cross-attention block: c:(B=8,N=8,C=512,H=32,W=32), RMSNorm over C, fused
KV projection (512->1024), one query per (batch, head) attending over the
N=8 token axis at each spatial position, then output projection (512->512).

Sharding: data-parallel over B — one batch element per NeuronCore (8 cores).

Per-core dataflow (feature-major: channels on partitions, the 1024 spatial
positions on the free dim):
  host prep : fold g into Wkv; qv = emb[q]@Wq+bq; fold qv and the 1/sqrt(64)
              logit scale into a per-batch matrix Wd (512x8) so attention
              logits come straight out of a matmul; k is never materialized.
  n loop    : DMA c[n]; square (DVE/ACT/GPSIMD); ssq and logits accumulate
              across n into persistent PSUM tiles via one-hot-padded
              stationary weights; vraw = Wv.T@cp -> fp16 in SBUF.
  epilogue  : batched softmax (one Sqrt + one Exp -> only 2 ACT table
              loads); softmax denominator via an exact-fp32 selection
              matmul; w~ = e*r/sums in fp16; per-head replication via
              broadcast DMAs from a DRAM bounce (all issued upfront);
              vw = vraw*w~ (DVE fp16); sum over n via identity-matmul
              PSUM accumulation; output projection + bias; DMA out in
              (C,H,W) layout.
Big matmuls run as float32r (fp32 data, 1 PE cycle/row).
"""

import numpy as np

import concourse.bass as bass
import concourse.bacc as bacc
import concourse.mybir as mybir
import concourse.tile as tile
from concourse.bass_utils import run_bass_kernel_spmd

F32 = mybir.dt.float32
F16 = mybir.dt.float16
F32R = mybir.dt.float32r
AF = mybir.ActivationFunctionType

B, N, C, H, W = 8, 8, 512, 32, 32
NH, HS = 8, 64
P = H * W           # 1024 spatial positions per core
NCC = C // 128      # 4 contraction chunks
EPS = 1e-6


def r32(ap):
    return ap if ap.dtype == F32R else ap.bitcast(F32R)


def build_program():
    nc = bacc.Bacc()

    c_d = nc.declare_dram_parameter("c", [N, C, H, W], F32R, isOutput=False)
    wv_d = nc.declare_dram_parameter("wv", [128, NCC, 512], F32R, isOutput=False)
    # zero-padded logit weights: [k, cc, n, n*8+i] nonzero only at column n*8+i
    wdz_d = nc.declare_dram_parameter("wdz", [128, NCC, N, N * NH], F32R,
                                      isOutput=False)
    oh_d = nc.declare_dram_parameter("onehot", [128, N, N], F32R, isOutput=False)
    sel_d = nc.declare_dram_parameter("sel", [N * NH, NH], F32, isOutput=False)
    r8_d = nc.declare_dram_parameter("r8sel", [NH, 2, NH * NH], F32, isOutput=False)
    s64_d = nc.declare_dram_parameter("sel64", [N * NH, N, NCC, 128], F16,
                                      isOutput=False)
    wo_d = nc.declare_dram_parameter("wout", [128, NCC, 512], F16, isOutput=False)
    id_d = nc.declare_dram_parameter("ident", [128, 128], F16, isOutput=False)
    bo_d = nc.declare_dram_parameter("bout", [128, NCC], F32, isOutput=False)
    out_d = nc.declare_dram_parameter("out", [C, H, W], F32, isOutput=True)

    with tile.TileContext(nc) as tc:
        with (
            tc.tile_pool(name="consts", bufs=1) as consts,
            tc.tile_pool(name="store", bufs=1) as store,
            tc.tile_pool(name="smalls", bufs=1) as smalls,
            tc.tile_pool(name="osb_pool", bufs=2) as osb_pool,
            tc.tile_pool(name="ps_stat", bufs=1, space="PSUM") as ps_stat,
            tc.tile_pool(name="ps_big", bufs=2, space="PSUM") as ps_big,
        ):
            # loop-critical consts first (tiny oh so PE can start early);
            # wv/wdz loads are emitted inside n=0 after the first cp chunks,
            # epilogue-only weights after the loop.
            wdz_sb = consts.tile([128, NCC, N, N * NH], F32R)
            nc.sync.dma_start(out=wdz_sb[:, 0], in_=wdz_d[:, 0])
            wv_sb = consts.tile([128, NCC, 512], F32R)
            nc.sync.dma_start(out=wv_sb[:, 0], in_=wv_d[:, 0])
            oh_sb = consts.tile([128, N, N], F32R)
            nc.sync.dma_start(out=oh_sb, in_=oh_d[:])
            sel_sb = consts.tile([N * NH, NH], F32)
            r8_sb = consts.tile([NH, 2, NH * NH], F32)
            s64_sb = consts.tile([N * NH, N, NCC, 128], F16)
            wo_sb = consts.tile([128, NCC, 512], F16)
            id_sb = consts.tile([128, 128], F16)
            bo_sb = consts.tile([128, NCC], F32)

            # persistent accumulators / stores
            vraw_all = store.tile([128, N, NCC, P], F16)   # 8 MiB
            o_sb = store.tile([128, NCC, P], F16)
            ssq_ps = ps_stat.tile([N, P], F32)             # 2 banks, whole loop
            draw_ps = ps_stat.tile([N * NH, P], F32)       # 2 banks, whole loop

            # ================= main loop over token index n =================
            cp_ctx = tc.tile_pool(name="cp_pool", bufs=2)
            cp_pool = cp_ctx.__enter__()
            sq_ctx = tc.tile_pool(name="sq_pool", bufs=2)
            sq_pool = sq_ctx.__enter__()
            for n in range(N):
                cp = cp_pool.tile([128, NCC, P], F32R)
                if n == 0:
                    # per-cc loads interleaved with the weights they unblock
                    for cc in range(NCC):
                        nc.sync.dma_start(
                            out=cp[:, cc, :],
                            in_=c_d[:].rearrange(
                                "n (cc k) h w -> n cc k (h w)", k=128)[n, cc],
                        )
                        if cc < NCC - 1:
                            nc.sync.dma_start(out=wdz_sb[:, cc + 1],
                                              in_=wdz_d[:, cc + 1])
                            nc.sync.dma_start(out=wv_sb[:, cc + 1],
                                              in_=wv_d[:, cc + 1])
                else:
                    for half in range(2):
                        nc.sync.dma_start(
                            out=cp[:, 2 * half:2 * half + 2, :],
                            in_=c_d[:].rearrange(
                                "n (hf cc k) h w -> n hf k cc (h w)",
                                hf=2, k=128)[n, half],
                        )

                def emit_draw(n=n, cp=cp):
                    for cc in range(NCC):
                        for h in range(2):
                            nc.tensor.matmul(
                                draw_ps[:, h * 512:(h + 1) * 512],
                                r32(wdz_sb[:, cc, n, :]),
                                r32(cp[:, cc, h * 512:(h + 1) * 512]),
                                start=(n == 0 and cc == 0),
                                stop=(n == N - 1 and cc == NCC - 1),
                            )

                def emit_vraw(n=n, cp=cp):
                    # cc-outer / h-inner: one weight load serves both halves
                    for ck in range(NCC):
                        v_ps = ps_big.tile([128, P], F32, tag="pair",
                                           name="v_ps")
                        for cc in range(NCC):
                            for h in range(2):
                                nc.tensor.matmul(
                                    v_ps[:, h * 512:(h + 1) * 512],
                                    r32(wv_sb[:, cc, ck * 128:(ck + 1) * 128]),
                                    r32(cp[:, cc, h * 512:(h + 1) * 512]),
                                    start=(cc == 0),
                                    stop=(cc == NCC - 1),
                                )
                        nc.scalar.copy(out=vraw_all[:, n, ck, :], in_=v_ps)

                def emit_ssq(n=n, cp=cp):
                    sq = sq_pool.tile([128, NCC, P], F32R, name="sq")
                    nc.vector.tensor_mul(out=sq[:, 0, :], in0=cp[:, 0, :], in1=cp[:, 0, :])
                    nc.gpsimd.tensor_mul(out=sq[:, 1, :], in0=cp[:, 1, :], in1=cp[:, 1, :])
                    nc.gpsimd.tensor_mul(out=sq[:, 2, :], in0=cp[:, 2, :], in1=cp[:, 2, :])
                    nc.gpsimd.tensor_mul(out=sq[:, 3, :], in0=cp[:, 3, :], in1=cp[:, 3, :])
                    # pre-sum the 4 chunks so ssq needs 2 matmuls/n, not 8
                    sqs = sq_pool.tile([128, P], F32R, name="sqs")
                    nc.vector.tensor_add(out=sqs, in0=sq[:, 0, :], in1=sq[:, 1, :])
                    nc.gpsimd.tensor_add(out=sq[:, 2, :], in0=sq[:, 2, :], in1=sq[:, 3, :])
                    nc.vector.tensor_add(out=sqs, in0=sqs, in1=sq[:, 2, :])
                    for h in range(2):
                        nc.tensor.matmul(
                            ssq_ps[:, h * 512:(h + 1) * 512],
                            r32(oh_sb[:, n, :]),
                            r32(sqs[:, h * 512:(h + 1) * 512]),
                            start=(n == 0),
                            stop=(n == N - 1),
                        )

                if n < N - 2:
                    # stats are epilogue-only: emit them last
                    emit_draw(); emit_vraw(); emit_ssq()
                elif n == N - 2:
                    # defer this vraw until after n=7's stats (loop tail)
                    emit_ssq(); emit_draw()
                    deferred_vraw = emit_vraw
                else:
                    # n=7: stats first, then both deferred vraws — the
                    # softmax chain hides under ~14us of vraw matmuls
                    emit_ssq(); emit_draw()
                    deferred_vraw(); emit_vraw()
            sq_ctx.__exit__(None, None, None)
            cp_ctx.__exit__(None, None, None)

            # ======================== epilogue ========================
            # epilogue-only weights (land during the loop's DMA slack)
            nc.sync.dma_start(out=sel_sb, in_=sel_d[:])
            nc.sync.dma_start(out=r8_sb, in_=r8_d[:])
            nc.sync.dma_start(out=s64_sb, in_=s64_d[:])
            nc.sync.dma_start(out=wo_sb, in_=wo_d[:])
            nc.sync.dma_start(out=id_sb, in_=id_d[:])
            nc.sync.dma_start(out=bo_sb, in_=bo_d[:])

            # r = 1/sqrt(ssq/C + eps)   (one Sqrt: single ACT table load)
            eps_sb = smalls.tile([N, 1], F32)
            nc.vector.memset(eps_sb, EPS)
            rt = smalls.tile([N, P], F32)
            nc.scalar.activation(out=rt, in_=ssq_ps, func=AF.Sqrt,
                                 scale=1.0 / C, bias=eps_sb)
            r_all = rt
            nc.vector.reciprocal_approx_fast(out=r_all, in_=rt)

            # rrep[n*8+i] = r_all[n] via selection matmul (exact fp32)
            rrep = smalls.tile([N * NH, P], F32)
            rr_ps = ps_big.tile([N * NH, P], F32, tag="pair")
            for h in range(2):
                nc.tensor.matmul(rr_ps[:, h * 512:(h + 1) * 512], r8_sb[:, 0, :],
                                 r_all[:, h * 512:(h + 1) * 512],
                                 start=True, stop=True)
            nc.scalar.copy(out=rrep, in_=rr_ps)
            # dots = draw * r ; e = exp(dots)
            e_all = smalls.tile([N * NH, P], F32)
            nc.vector.tensor_mul(out=e_all, in0=draw_ps, in1=rrep)
            nc.scalar.activation(out=e_all, in_=e_all, func=AF.Exp)

            # softmax denominator: sums[i] = sum_n e[n*8+i]
            # exact-fp32 matmul (small, 4 cyc/row is fine)
            rsum = smalls.tile([NH, P], F32)
            s_ps = ps_big.tile([NH, P], F32, tag="pair")
            for h in range(2):
                nc.tensor.matmul(
                    s_ps[:, h * 512:(h + 1) * 512], sel_sb,
                    e_all[:, h * 512:(h + 1) * 512],
                    start=True, stop=True,
                )
            nc.vector.reciprocal_approx_fast(out=rsum, in_=s_ps)
            srep = smalls.tile([N * NH, P], F32)
            sr_ps = ps_big.tile([N * NH, P], F32, tag="pair")
            for h in range(2):
                nc.tensor.matmul(sr_ps[:, h * 512:(h + 1) * 512], r8_sb[:, 1, :],
                                 rsum[:, h * 512:(h + 1) * 512],
                                 start=True, stop=True)
            nc.scalar.copy(out=srep, in_=sr_ps)

            # w~ = e * r / sums  -> fp16, bounce to DRAM for broadcasts
            nc.vector.tensor_mul(out=e_all, in0=e_all, in1=rrep)
            wt = smalls.tile([N * NH, P], F16)
            nc.vector.tensor_mul(out=wt, in0=e_all, in1=srep)

            with (
                tc.tile_pool(name="wrep_pool", bufs=4) as wrep_pool,
                tc.tile_pool(name="vw_pool", bufs=4) as vw_pool,
            ):
                # o = sum_n vraw * w~rep via identity-matmul PSUM accumulation;
                # per-head replication via selection matmuls from wt (on-chip)
                for ck in range(NCC):
                    # o-accumulator reuses the (now idle) stats PSUM banks so
                    # ps_big's 4 slots stay free for the wrep pipeline
                    on_ps = ps_stat.tile(
                        [128, P], F32, name=f"on_ps_{ck}", tag="ssq_ps")
                    for n in range(N):
                        vw = vw_pool.tile([128, P], F16)
                        if n % 3 == 2:
                            wr_ps = ps_stat.tile([128, P], F32, tag="draw_ps",
                                                 name="wr_ps_d")
                        else:
                            wr_ps = ps_big.tile([128, P], F32, tag="pair",
                                                name="wr_ps")
                        for h in range(2):
                            nc.tensor.matmul(
                                wr_ps[:, h * 512:(h + 1) * 512],
                                s64_sb[:, n, ck, :],
                                wt[:, h * 512:(h + 1) * 512],
                                start=True, stop=True)
                        nc.vector.tensor_mul(
                            out=vw, in0=vraw_all[:, n, ck, :], in1=wr_ps)
                        for h in range(2):
                            nc.tensor.matmul(
                                on_ps[:, h * 512:(h + 1) * 512],
                                id_sb,
                                vw[:, h * 512:(h + 1) * 512],
                                start=(n == 0),
                                stop=(n == N - 1),
                            )
                    for h in range(2):
                        nc.scalar.copy(
                            out=o_sb[:, ck, h * 512:(h + 1) * 512],
                            in_=on_ps[:, h * 512:(h + 1) * 512]
                        )

                # out = Wout.T @ o + bout
                for do in range(NCC):
                    ot_sb = osb_pool.tile([128, P], F32)
                    ot_ps = ps_big.tile([128, P], F32, tag="pair")
                    for h in range(2):
                        for di in range(NCC):
                            nc.tensor.matmul(
                                ot_ps[:, h * 512:(h + 1) * 512],
                                wo_sb[:, di, do * 128:(do + 1) * 128],
                                o_sb[:, di, h * 512:(h + 1) * 512],
                                start=(di == 0),
                                stop=(di == NCC - 1),
                            )
                    nc.scalar.activation(
                        out=ot_sb, in_=ot_ps,
                        func=AF.Identity, bias=bo_sb[:, do:do + 1],
                    )
                    nc.sync.dma_start(
                        out=out_d[:].rearrange(
                            "(do k) h w -> do k (h w)", k=128)[do],
                        in_=ot_sb,
                    )

    nc.finalize()
    return nc


_CACHE = {}


def _get_nc():
    if "nc" not in _CACHE:
        _CACHE["nc"] = build_program()
    return _CACHE["nc"]


def _prep_inputs(q, c, emb, Wq, bq, Wkv, Wout, bout, g):
    q = np.asarray(q)
    c = np.asarray(c, dtype=np.float32)
    emb = np.asarray(emb, dtype=np.float32)
    Wq = np.asarray(Wq, dtype=np.float32)
    bq = np.asarray(bq, dtype=np.float32)
    Wkv = np.asarray(Wkv, dtype=np.float32)
    Wout = np.asarray(Wout, dtype=np.float32)
    bout = np.asarray(bout, dtype=np.float32)
    g = np.asarray(g, dtype=np.float32)

    qv = emb[q] @ Wq + bq                                   # (B, 512)
    qvs = qv.reshape(B, NH, HS).astype(np.float32) * np.float32(HS ** -0.5)
    Wkv_g = (g[:, None] * Wkv).astype(np.float32)
    Wk3 = Wkv_g[:, :C].reshape(C, NH, HS)
    Wv = np.ascontiguousarray(Wkv_g[:, C:])                 # (512, 512)
    Wd = np.einsum('chs,bhs->bch', Wk3, qvs).astype(np.float32)  # (B, 512, 8)

    wv_host = np.ascontiguousarray(
        Wv.reshape(NCC, 128, 512).transpose(1, 0, 2))       # [k, cc, dv]
    # zero-padded draw weights: [b, k, cc, n, m] = Wd at m = n*8+i
    wdz = np.zeros((B, 128, NCC, N, N * NH), np.float32)
    wd4 = Wd.reshape(B, NCC, 128, NH).transpose(0, 2, 1, 3)  # [b, k, cc, i]
    for n in range(N):
        wdz[:, :, :, n, n * NH:(n + 1) * NH] = wd4
    wout_host = np.ascontiguousarray(
        Wout.reshape(NCC, 128, 512).transpose(1, 0, 2)).astype(np.float16)
    onehot = np.zeros((128, N, N), np.float32)
    for n in range(N):
        onehot[:, n, n] = 1.0
    sel = np.zeros((N * NH, NH), np.float32)
    for n in range(N):
        for i in range(NH):
            sel[n * NH + i, i] = 1.0
    # r8sel[:, 0]: rrep (out row n*8+i <- r row n); r8sel[:, 1]: srep (<- rsum row i)
    r8sel = np.zeros((NH, 2, NH * NH), np.float32)
    for n in range(N):
        for i in range(NH):
            r8sel[n, 0, n * NH + i] = 1.0
            r8sel[i, 1, n * NH + i] = 1.0
    # sel64[kk, n, ck, m] = 1 iff kk == n*8 + 2*ck + m//64
    sel64 = np.zeros((N * NH, N, NCC, 128), np.float16)
    for n in range(N):
        for ck in range(NCC):
            for j in range(2):
                sel64[n * NH + 2 * ck + j, n, ck, j * 64:(j + 1) * 64] = 1.0
    ident = np.eye(128, dtype=np.float16)
    bout_host = np.ascontiguousarray(bout.reshape(NCC, 128).T)  # [k, do]

    in_maps = []
    for b in range(B):
        in_maps.append({
            "c": np.ascontiguousarray(c[b]),
            "wv": wv_host,
            "wdz": np.ascontiguousarray(wdz[b]),
            "onehot": onehot,
            "sel": sel,
            "r8sel": r8sel,
            "sel64": sel64,
            "wout": wout_host,
            "ident": ident,
            "bout": bout_host,
        })
    return in_maps


def kernel(**inputs) -> np.ndarray:
    nc = _get_nc()
    in_maps = _prep_inputs(**inputs)
    res = run_bass_kernel_spmd(nc, in_maps, list(range(B)))
    return np.stack([res.results[b]["out"] for b in range(B)], axis=0)


if __name__ == "__main__":
    nc = build_program()
    print("program built ok")
